# revision 31
# baseline (speedup 1.0000x reference)
"""Multi-head self-attention with RoPE on 8 Trainium2 NeuronCores.

Sharding: core c = (b, g) with b = c // 4 (batch of 2), g = c % 4 (head
group of 4 heads out of 16). Each core computes Q/K/V projections for its
4 heads on its batch, RoPE, causal attention, producing a context slab
(256 features x 2048 tokens). Each core then applies the FULL output
projection restricted to its own 256 ctx features, producing a bf16
PARTIAL output (1024, 2048); the host sums the 4 partials per batch
during unsharding. No collectives at all — v4's 8 AllGathers cost
60-100us of wall (8-25us each, serialized behind a ~35us rank-skew
rendezvous barrier) and an irreducible tail.

v4 notes:
- Projection and attention INTERLEAVED per 512-token block:
  proj(0) attn(0) proj(1) attn(1) proj(2) attn(2) [op0] proj(3) attn(3)
  [op1] op2 op3. PE stays busy (no HAM re-throttle), AGs post early.
- 8 AllGathers of 128KB (one per q-block x head-pair, v2 shape): the
  v3 256KB combined AGs ran 25-28us each; small ones measured 5-16us.
- AG trigger chain never head-of-line blocks: ag_in stores ride the
  Vector queue (data-local), AG-output reloads ride Sync; GpSimd holds
  only const loads + collective triggers, so trigger(i+1) doesn't wait
  for reload(i) (v3 serialized the whole chain through gpsimd).
- Tiny warmup collective at kernel start absorbs the one-time ~36-47us
  cc rendezvous barrier behind proj(0).
- Off-diagonal key-chunk PAIRS share one bf16 PSUM bank [128,1024]:
  one DVE Schraudolph tensor_scalar per pair instead of per chunk
  (DVE tensor_scalar from PSUM runs at 1x; halve the op count).
- Single-trigger bulk weight DMAs; x prefetched per 512-block; rope
  swap DMAs split between GpSimd and Scalar queues.

Self-contained: hardcodes all shapes; builds and compiles the SPMD Bass
program once per process.
"""
import os
import numpy as np

import concourse.bass as bass
import concourse.mybir as mybir
import concourse.tile as tile
from concourse import bacc
from concourse.bass_utils import run_bass_kernel_spmd

B, S, D, H, DK = 2, 2048, 1024, 16, 64
NF = DK // 2            # 32 rotary frequencies
HPC = 4                 # heads per core
GF = HPC * DK           # 256 features per core
NCORES = 8
THETA = 10000.0
F32 = mybir.dt.float32
BF16 = mybir.dt.bfloat16
AF = mybir.ActivationFunctionType

_CACHE: dict = {}


def _emit(nc: bacc.Bacc, debug: bool = False):
    xT = nc.dram_tensor("xT", [D, S], BF16, kind="ExternalInput").ap()
    wqT = nc.dram_tensor("wqT", [D, GF], BF16, kind="ExternalInput").ap()
    wkT = nc.dram_tensor("wkT", [D, GF], BF16, kind="ExternalInput").ap()
    wvT = nc.dram_tensor("wvT", [D, GF], BF16, kind="ExternalInput").ap()
    woT = nc.dram_tensor("woT", [GF, D], BF16, kind="ExternalInput").ap()
    cs_d = nc.dram_tensor("cs", [128, S], BF16, kind="ExternalInput").ap()
    ss_d = nc.dram_tensor("ss", [128, S], BF16, kind="ExternalInput").ap()
    ones_d = nc.dram_tensor("ones", [128, 64], BF16, kind="ExternalInput").ap()
    tri_d = nc.dram_tensor("tri", [128, 128], BF16, kind="ExternalInput").ap()
    out_d = nc.dram_tensor("out", [D, S], BF16, kind="ExternalOutput").ap()
    dbg = {}
    if debug:
        for nm in ("dbg_qt0", "dbg_qt1", "dbg_kt0", "dbg_kt1"):
            dbg[nm] = nc.dram_tensor(nm, [128, S], BF16, kind="ExternalOutput").ap()
        dbg["dbg_v"] = nc.dram_tensor("dbg_v", [128, (S // 128) * 260], BF16,
                                      kind="ExternalOutput").ap()
        for p in range(2):
            dbg[f"dbg_ctx{p}"] = nc.dram_tensor(f"dbg_ctx{p}", [128, S], BF16,
                                                kind="ExternalOutput").ap()

    NKT = D // 128       # 8 contraction tiles for projections
    NQB = S // 512       # 4 query 512-blocks

    inv_sqrt_dk = float(1.0 / np.sqrt(DK))
    # Schraudolph exp on DVE: bf16(2^e) bit pattern == int16(round(
    # x*log2(e)*128 + 127*128 + c)); one tensor_scalar mult+add.
    sch_a = float(np.log2(np.e) * 128.0 * inv_sqrt_dk)
    sch_b = float(127 * 128 - 7.33)

    with tile.TileContext(nc) as tc:
        with (
            tc.tile_pool(name="singles", bufs=1) as singles,
            tc.tile_pool(name="dram", bufs=1, space="DRAM") as dram,
        ):
            # ---- resident tiles; one bulk DMA each, first-needed-first ----
            wq_sb = singles.tile([128, NKT, GF], BF16, tag="wq")
            wk_sb = singles.tile([128, NKT, GF], BF16, tag="wk")
            wv_sb = singles.tile([128, NKT, GF], BF16, tag="wv")
            wo_sb = singles.tile([128, 2, D], BF16, tag="wo")
            for h in range(4):
                nc.scalar.dma_start(
                    out=wq_sb[:, 2 * h:2 * (h + 1), :],
                    in_=wqT[256 * h:256 * (h + 1), :].rearrange(
                        "(k p) n -> p k n", p=128))
            nc.scalar.dma_start(out=wk_sb[:], in_=wkT.rearrange("(k p) n -> p k n", p=128))
            nc.scalar.dma_start(out=wv_sb[:], in_=wvT.rearrange("(k p) n -> p k n", p=128))
            nc.scalar.dma_start(out=wo_sb[:], in_=woT.rearrange("(k p) n -> p k n", p=128))
            cs_sb = singles.tile([128, S], BF16, tag="cs")
            ss_sb = singles.tile([128, S], BF16, tag="ss")
            ones_sb = singles.tile([128, 64], BF16, tag="ones")
            tri_sb = singles.tile([128, 128], BF16, tag="tri")
            nc.gpsimd.dma_start(out=cs_sb[:, 0:512], in_=cs_d[:, 0:512])
            nc.gpsimd.dma_start(out=ss_sb[:, 0:512], in_=ss_d[:, 0:512])
            nc.gpsimd.dma_start(out=cs_sb[:, 512:S], in_=cs_d[:, 512:S])
            nc.gpsimd.dma_start(out=ss_sb[:, 512:S], in_=ss_d[:, 512:S])
            nc.gpsimd.dma_start(out=ones_sb[:], in_=ones_d[:])
            nc.gpsimd.dma_start(out=tri_sb[:], in_=tri_d[:])

            # roped Q^T / K^T: 2 tiles each, rows = [headA(64) | headB(64)],
            # within each head block [x0(32) | x1(32)]
            qt = [singles.tile([128, S], BF16, tag=f"qt{m}", name=f"qt{m}") for m in range(2)]
            kt = [singles.tile([128, S], BF16, tag=f"kt{m}", name=f"kt{m}") for m in range(2)]
            # V with per-head ones column: head h occupies cols 65h..65h+63,
            # col 65h+64 is 1.0 (softmax denominator rides the PV matmul)
            NVT = S // 128
            v_sb = singles.tile([128, NVT, 4 * 65], BF16, tag="v")
            nc.vector.tensor_copy(
                v_sb.rearrange("p t (h e) -> p t h e", h=4)[:, :, :, 64:65],
                ones_sb.rearrange("p (t h) -> p t h", t=NVT)[:, :, :, None])
            # context per p-pair: rows [headA(64) | headB(64)]
            ctx2_sb = [singles.tile([128, S], BF16, tag=f"ctx{p}", name=f"ctx{p}")
                       for p in range(2)]

            with (
                tc.tile_pool(name="xin", bufs=4) as xin,
                tc.tile_pool(name="qkraw", bufs=3) as qkraw,
                tc.tile_pool(name="ropetmp", bufs=4) as ropetmp,
                tc.tile_pool(name="probs", bufs=6) as probspool,
                tc.tile_pool(name="recips", bufs=2) as recips,
                tc.tile_pool(name="ctxu", bufs=3) as ctxupool,
                tc.tile_pool(name="outsb", bufs=2) as outsb,
                tc.tile_pool(name="ps_pj", bufs=2, space="PSUM") as ps_pj,
                tc.tile_pool(name="ps_sc", bufs=4, space="PSUM") as ps_sc,
                tc.tile_pool(name="ps_ctx", bufs=2, space="PSUM") as ps_ctx,
            ):
                ps_bc = ps_sc       # pbc shares the ps_sc ring (same shape)

                # prefetch all x blocks up front: the sync queue later
                # carries AG-output reloads (which wait on collectives) and
                # must never have an x load queued behind one.
                xts = []
                for j in range(NQB):
                    xt_ = xin.tile([128, NKT, 512], BF16, name="xt")
                    nh = 4 if j == 0 else 1
                    for h in range(nh):
                        k0, k1 = (8 // nh) * h, (8 // nh) * (h + 1)
                        nc.sync.dma_start(
                            out=xt_[:, k0:k1, :],
                            in_=xT[128 * k0:128 * k1,
                                   512 * j:512 * (j + 1)].rearrange(
                                "(k p) n -> p k n", p=128))
                    xts.append(xt_)

                def emit_proj(j):
                    csl = slice(512 * j, 512 * (j + 1))
                    xt_ = xts[j]
                    # Q^T and K^T tiles: out (128 qdim, 512 tok)
                    for w_sb, raw_dst in ((wq_sb, qt), (wk_sb, kt)):
                        for m in range(2):
                            raw = qkraw.tile([128, 512], BF16, tag="raw", name="raw")
                            pq = ps_pj.tile([128, 512], F32, tag="pp", name="pq")
                            for k in range(NKT):
                                nc.tensor.matmul(
                                    pq[:], w_sb[:, k, 128 * m:128 * (m + 1)],
                                    xt_[:, k, :],
                                    start=(k == 0), stop=(k == NKT - 1))
                            nc.scalar.copy(out=raw[:], in_=pq[:])
                            # rope: dst = raw*cs + swap(raw)*ss; partner is
                            # 32 partitions away; 4 strided SBUF DMAs split
                            # across the gpsimd and scalar queues.
                            sw = ropetmp.tile([128, 512], BF16, tag="sw", name="sw")
                            for blk in range(2):
                                nc.gpsimd.dma_start(
                                    out=sw[64 * blk:64 * blk + 32, :],
                                    in_=raw[64 * blk + 32:64 * blk + 64, :])
                                nc.gpsimd.dma_start(
                                    out=sw[64 * blk + 32:64 * blk + 64, :],
                                    in_=raw[64 * blk:64 * blk + 32, :])
                            t1 = ropetmp.tile([128, 512], BF16, tag="t1", name="t1")
                            nc.vector.tensor_mul(t1[:], raw[:], cs_sb[:, csl])
                            nc.vector.tensor_mul(sw[:], sw[:], ss_sb[:, csl])
                            nc.vector.tensor_add(raw_dst[m][:, csl], t1[:], sw[:])
                    # V tiles: 2 token-tiles per PSUM bank, scattered into
                    # the 65-stride layout
                    for sp in range(2):
                        pv = ps_pj.tile([128, 512], F32, tag="pp", name="pv")
                        for i in range(2):
                            s_ = 2 * sp + i
                            for k in range(NKT):
                                nc.tensor.matmul(
                                    pv[:, 256 * i:256 * (i + 1)],
                                    xt_[:, k, 128 * s_:128 * (s_ + 1)],
                                    wv_sb[:, k, :],
                                    start=(k == 0), stop=(k == NKT - 1))
                        vt = 4 * j + 2 * sp
                        dst = v_sb[:, vt:vt + 2, :].rearrange(
                            "p t (h e) -> p t h e", h=4)[:, :, :, 0:64]
                        nc.vector.tensor_copy(
                            dst, pv[:].rearrange("p (t h e) -> p t h e", t=2, h=4))

                def emit_attn(qj, ps=(0, 1)):
                    qsl = slice(512 * qj, 512 * (qj + 1))
                    # diagonal chunks (windowed; i=0 covers the full 512 q so
                    # start=True initializes every PSUM element), then the
                    # fully-causal chunks in PAIRS sharing a bf16 PSUM bank.
                    diag_list = [(4 * qj + i, 128 * i) for i in range(4)]
                    off_list = list(range(4 * qj))
                    # interleave the ACT-heavy diag chunks (2 ACT exps each)
                    # among the off-diag ones (ACT/DVE split) so the scalar
                    # engine isn't burst-loaded; unit 0 must stay the
                    # full-window diag (start=True covers all 512 queries).
                    sched = [diag_list[0]]
                    per = -(-len(off_list) // 3) if off_list else 0
                    oi = 0
                    for dch in diag_list[1:]:
                        sched += [(c, 0) for c in off_list[oi:oi + per]]
                        oi += per
                        sched.append(dch)
                    sched += [(c, 0) for c in off_list[oi:]]
                    nunits = len(sched)
                    for p in ps:
                        pctx = [ps_ctx.tile([65, 512], F32, tag="ctx", name="pctx")
                                for _ in range(2)]
                        for ui, (ch, w0) in enumerate(sched):
                            diag = ch >= 4 * qj
                            pscs, prbs = [], []
                            for hh in range(2):
                                psc = ps_sc.tile([128, 512], F32, tag="ps", name="psc")
                                rsl = slice(64 * hh, 64 * (hh + 1))
                                nc.tensor.matmul(
                                    psc[:, w0:512],
                                    kt[p][rsl, 128 * ch:128 * (ch + 1)],
                                    qt[p][rsl, 512 * qj + w0:512 * (qj + 1)],
                                    start=True, stop=True)
                                pscs.append(psc)
                            for hh in range(2):
                                if not diag and hh == 1:
                                    pri = probspool.tile([128, 512], mybir.dt.int16,
                                                         tag="pri", name="pri")
                                    nc.vector.tensor_scalar(
                                        out=pri[:], in0=pscs[hh][:],
                                        scalar1=sch_a, scalar2=sch_b,
                                        op0=mybir.AluOpType.mult,
                                        op1=mybir.AluOpType.add)
                                    probs = pri[:].bitcast(BF16)
                                else:
                                    probs = probspool.tile([128, 512], BF16,
                                                           tag="prb", name="prb")
                                    nc.scalar.activation(out=probs[:, w0:512],
                                                         in_=pscs[hh][:, w0:512],
                                                         func=AF.Exp,
                                                         scale=inv_sqrt_dk)
                                    if diag:
                                        sl = probs[:, w0:w0 + 128]
                                        nc.vector.tensor_mul(sl, sl, tri_sb[:])
                                prbs.append(probs)
                            for hh in range(2):
                                h65 = 65 * (2 * p + hh)
                                nc.tensor.matmul(
                                    pctx[hh][:, w0:512],
                                    v_sb[:, ch, h65:h65 + 65],
                                    prbs[hh][:, w0:512],
                                    start=(ui == 0), stop=(ui == nunits - 1))
                        # normalize: denominator rode the PV matmul as row 64
                        recip = recips.tile([128, 1024], F32, tag="recip", name="recip")
                        recipr = recips.tile([128, 1024], BF16, tag="recipr", name="recipr")
                        ctxus = []
                        for hh in range(2):
                            ctxu = ctxupool.tile([65, 512], F32, tag="ctxu", name="ctxu")
                            nc.scalar.copy(out=ctxu[:], in_=pctx[hh][:])
                            nc.vector.reciprocal_approx_fast(
                                out=recip[0:65, 512 * hh:512 * (hh + 1)],
                                in_=ctxu[:])
                            ctxus.append(ctxu)
                        nc.vector.tensor_copy(recipr[64:65, :], recip[64:65, :])
                        for hh in range(2):
                            pbc = ps_bc.tile([128, 512], F32, tag="ps", name="pbc")[0:64, :]
                            nc.tensor.matmul(
                                pbc[:], ones_sb[64:65, 0:64],
                                recipr[64:65, 512 * hh:512 * (hh + 1)],
                                start=True, stop=True)
                            nc.vector.tensor_mul(
                                ctx2_sb[p][64 * hh:64 * (hh + 1), qsl],
                                ctxus[hh][0:64, :], pbc[:])

                def emit_outproj(qjo):
                    qsl = slice(512 * qjo, 512 * (qjo + 1))
                    stago = outsb.tile([128, 8, 512], BF16, tag="ot", name="stago")
                    for m in range(8):
                        po = ps_sc.tile([128, 512], F32, tag="ps", name="po")
                        for k in range(2):
                            nc.tensor.matmul(
                                po[:], wo_sb[:, k, 128 * m:128 * (m + 1)],
                                ctx2_sb[k][:, qsl],
                                start=(k == 0), stop=(k == 1))
                        # PSUM->SBUF bf16 cast copies: ACT (queue freed of
                        # swap DMAs), DVE takes every fourth for balance
                        if m % 4 == 3:
                            nc.vector.tensor_copy(stago[:, m, :], po[:])
                        else:
                            nc.scalar.copy(out=stago[:, m, :], in_=po[:])
                    for h in range(2):
                        nc.sync.dma_start(
                            out=out_d[512 * h:512 * (h + 1), qsl].rearrange(
                                "(m p) n -> p m n", p=128),
                            in_=stago[:, 4 * h:4 * (h + 1), :])

                # interleaved schedule: proj(j) unlocks attn(j); outproj
                # pipelined well behind its AG.
                emit_proj(0)
                emit_attn(0)
                emit_proj(1)
                emit_attn(1, ps=(0,))
                emit_outproj(0)
                emit_attn(1, ps=(1,))
                emit_proj(2)
                emit_attn(2, ps=(0,))
                emit_outproj(1)
                emit_attn(2, ps=(1,))
                emit_proj(3)
                emit_attn(3, ps=(0,))
                emit_outproj(2)
                emit_attn(3, ps=(1,))
                emit_outproj(3)

                if debug:
                    for m in range(2):
                        nc.sync.dma_start(out=dbg[f"dbg_qt{m}"][:], in_=qt[m][:])
                        nc.sync.dma_start(out=dbg[f"dbg_kt{m}"][:], in_=kt[m][:])
                    nc.sync.dma_start(out=dbg["dbg_v"][:],
                                      in_=v_sb.rearrange("p t e -> p (t e)"))
                    for p in range(2):
                        nc.sync.dma_start(out=dbg[f"dbg_ctx{p}"][:], in_=ctx2_sb[p][:])


def _build(debug: bool = False):
    nc = bacc.Bacc("TRN2", target_bir_lowering=False, debug=False, num_devices=NCORES)
    _emit(nc, debug=debug)
    nc.compile()
    return nc


def _perm_rows(g: int) -> np.ndarray:
    rows = []
    for l in range(HPC):
        h = HPC * g + l
        rows += [DK * h + d for d in range(0, DK, 2)]
        rows += [DK * h + d for d in range(1, DK, 2)]
    return np.asarray(rows)


def _wo_feat_perm(g: int) -> np.ndarray:
    # ctx2 slab feature order: slab p rows = [head 4g+2p (64) | head 4g+2p+1]
    perm = []
    for p in range(2):
        for hh in range(2):
            h = 4 * g + 2 * p + hh
            perm += [DK * h + d for d in range(DK)]
    return np.asarray(perm)


def kernel(x, token_positions, Wq, Wk, Wv, Wo):
    bf = mybir.dt.np(BF16)
    x = np.asarray(x, dtype=np.float32)
    Wq = np.asarray(Wq, dtype=np.float32)
    Wk = np.asarray(Wk, dtype=np.float32)
    Wv = np.asarray(Wv, dtype=np.float32)
    Wo = np.asarray(Wo, dtype=np.float32)
    pos = np.asarray(token_positions).astype(np.float64)

    debug = os.environ.get("KERNEL_DEBUG", "0") == "1"
    if "nc" not in _CACHE:
        _CACHE["nc"] = _build(debug=debug)
    nc = _CACHE["nc"]

    inv_freq = np.exp(np.arange(0, DK, 2, dtype=np.float32) * (-np.log(THETA) / DK)).astype(np.float64)
    ang = pos[:, None] * inv_freq[None, :]              # (S, 32)
    cos_t = np.cos(ang).astype(np.float32).T            # (32, S)
    sin_t = np.sin(ang).astype(np.float32).T
    fi = np.arange(128) % NF
    half = (np.arange(128) // NF) % 2
    CS = np.ascontiguousarray(cos_t[fi, :]).astype(bf)
    SS = np.ascontiguousarray(
        np.where(half[:, None] == 0, -sin_t[fi, :], sin_t[fi, :])).astype(bf)
    ONES = np.ones((128, 64), dtype=np.float32).astype(bf)
    TRI = np.triu(np.ones((128, 128), dtype=np.float32)).astype(bf)  # keep k<=q

    in_maps = []
    for c in range(NCORES):
        b, g = divmod(c, 4)
        pr = _perm_rows(g)
        in_maps.append({
            "xT": np.ascontiguousarray(x[b].T).astype(bf),
            "wqT": np.ascontiguousarray(Wq[pr].T).astype(bf),
            "wkT": np.ascontiguousarray(Wk[pr].T).astype(bf),
            "wvT": np.ascontiguousarray(Wv[GF * g:GF * (g + 1)].T).astype(bf),
            "woT": np.ascontiguousarray(Wo[:, _wo_feat_perm(g)].T).astype(bf),
            "cs": CS, "ss": SS, "ones": ONES, "tri": TRI,
        })

    trace = os.environ.get("KERNEL_TRACE", "0") == "1"
    res = run_bass_kernel_spmd(nc, in_maps, list(range(NCORES)), trace=trace)
    _CACHE["last_result"] = res

    acc = np.zeros((B, D, S), dtype=np.float32)
    for c in range(NCORES):
        b, g = divmod(c, 4)
        acc[b] += res.results[c]["out"].astype(np.float32)
    return np.ascontiguousarray(acc.transpose(0, 2, 1))


# revision 32
# speedup vs baseline: 1.0535x; 1.0535x over previous
"""Multi-head self-attention with RoPE on 8 Trainium2 NeuronCores.

Sharding: core c = (b, g) with b = c // 4 (batch of 2), g = c % 4 (head
group of 4 heads out of 16). Each core computes Q/K/V projections for its
4 heads on its batch, RoPE, causal attention, producing a context slab
(256 features x 2048 tokens). Each core then applies the FULL output
projection restricted to its own 256 ctx features, producing a bf16
PARTIAL output (1024, 2048); the host sums the 4 partials per batch
during unsharding. No collectives at all — v4's 8 AllGathers cost
60-100us of wall (8-25us each, serialized behind a ~35us rank-skew
rendezvous barrier) and an irreducible tail.

v4 notes:
- Projection and attention INTERLEAVED per 512-token block:
  proj(0) attn(0) proj(1) attn(1) proj(2) attn(2) [op0] proj(3) attn(3)
  [op1] op2 op3. PE stays busy (no HAM re-throttle), AGs post early.
- 8 AllGathers of 128KB (one per q-block x head-pair, v2 shape): the
  v3 256KB combined AGs ran 25-28us each; small ones measured 5-16us.
- AG trigger chain never head-of-line blocks: ag_in stores ride the
  Vector queue (data-local), AG-output reloads ride Sync; GpSimd holds
  only const loads + collective triggers, so trigger(i+1) doesn't wait
  for reload(i) (v3 serialized the whole chain through gpsimd).
- Tiny warmup collective at kernel start absorbs the one-time ~36-47us
  cc rendezvous barrier behind proj(0).
- Off-diagonal key-chunk PAIRS share one bf16 PSUM bank [128,1024]:
  one DVE Schraudolph tensor_scalar per pair instead of per chunk
  (DVE tensor_scalar from PSUM runs at 1x; halve the op count).
- Single-trigger bulk weight DMAs; x prefetched per 512-block; rope
  swap DMAs split between GpSimd and Scalar queues.

Self-contained: hardcodes all shapes; builds and compiles the SPMD Bass
program once per process.
"""
import os
import numpy as np

import concourse.bass as bass
import concourse.mybir as mybir
import concourse.tile as tile
from concourse import bacc
from concourse.bass_utils import run_bass_kernel_spmd

B, S, D, H, DK = 2, 2048, 1024, 16, 64
NF = DK // 2            # 32 rotary frequencies
HPC = 4                 # heads per core
GF = HPC * DK           # 256 features per core
NCORES = 8
THETA = 10000.0
F32 = mybir.dt.float32
BF16 = mybir.dt.bfloat16
AF = mybir.ActivationFunctionType

_CACHE: dict = {}


def _emit(nc: bacc.Bacc, debug: bool = False):
    xT = nc.dram_tensor("xT", [D, S], BF16, kind="ExternalInput").ap()
    wqT = nc.dram_tensor("wqT", [D, GF], BF16, kind="ExternalInput").ap()
    wkT = nc.dram_tensor("wkT", [D, GF], BF16, kind="ExternalInput").ap()
    wvT = nc.dram_tensor("wvT", [D, GF], BF16, kind="ExternalInput").ap()
    woT = nc.dram_tensor("woT", [GF, D], BF16, kind="ExternalInput").ap()
    cs_d = nc.dram_tensor("cs", [128, S], BF16, kind="ExternalInput").ap()
    ss_d = nc.dram_tensor("ss", [128, S], BF16, kind="ExternalInput").ap()
    ones_d = nc.dram_tensor("ones", [128, 64], BF16, kind="ExternalInput").ap()
    tri_d = nc.dram_tensor("tri", [128, 128], BF16, kind="ExternalInput").ap()
    out_d = nc.dram_tensor("out", [D, S], BF16, kind="ExternalOutput").ap()
    dbg = {}
    if debug:
        for nm in ("dbg_qt0", "dbg_qt1", "dbg_kt0", "dbg_kt1"):
            dbg[nm] = nc.dram_tensor(nm, [128, S], BF16, kind="ExternalOutput").ap()
        dbg["dbg_v"] = nc.dram_tensor("dbg_v", [128, (S // 128) * 260], BF16,
                                      kind="ExternalOutput").ap()
        for p in range(2):
            dbg[f"dbg_ctx{p}"] = nc.dram_tensor(f"dbg_ctx{p}", [128, S], BF16,
                                                kind="ExternalOutput").ap()

    NKT = D // 128       # 8 contraction tiles for projections
    NQB = S // 512       # 4 query 512-blocks

    inv_sqrt_dk = float(1.0 / np.sqrt(DK))
    # Schraudolph exp on DVE: bf16(2^e) bit pattern == int16(round(
    # x*log2(e)*128 + 127*128 + c)); one tensor_scalar mult+add.
    sch_a = float(np.log2(np.e) * 128.0 * inv_sqrt_dk)
    sch_b = float(127 * 128 - 7.33)

    with tile.TileContext(nc) as tc:
        with (
            tc.tile_pool(name="singles", bufs=1) as singles,
            tc.tile_pool(name="dram", bufs=1, space="DRAM") as dram,
        ):
            # ---- resident tiles; one bulk DMA each, first-needed-first ----
            wq_sb = singles.tile([128, NKT, GF], BF16, tag="wq")
            wk_sb = singles.tile([128, NKT, GF], BF16, tag="wk")
            wv_sb = singles.tile([128, NKT, GF], BF16, tag="wv")
            wo_sb = singles.tile([128, 2, D], BF16, tag="wo")
            for h in range(4):
                nc.scalar.dma_start(
                    out=wq_sb[:, 2 * h:2 * (h + 1), :],
                    in_=wqT[256 * h:256 * (h + 1), :].rearrange(
                        "(k p) n -> p k n", p=128))
            nc.scalar.dma_start(out=wk_sb[:], in_=wkT.rearrange("(k p) n -> p k n", p=128))
            nc.scalar.dma_start(out=wv_sb[:], in_=wvT.rearrange("(k p) n -> p k n", p=128))
            nc.scalar.dma_start(out=wo_sb[:], in_=woT.rearrange("(k p) n -> p k n", p=128))
            cs_sb = singles.tile([128, S], BF16, tag="cs")
            ss_sb = singles.tile([128, S], BF16, tag="ss")
            ones_sb = singles.tile([128, 64], BF16, tag="ones")
            tri_sb = singles.tile([128, 128], BF16, tag="tri")
            nc.gpsimd.dma_start(out=cs_sb[:, 0:512], in_=cs_d[:, 0:512])
            nc.gpsimd.dma_start(out=ss_sb[:, 0:512], in_=ss_d[:, 0:512])
            nc.gpsimd.dma_start(out=cs_sb[:, 512:S], in_=cs_d[:, 512:S])
            nc.gpsimd.dma_start(out=ss_sb[:, 512:S], in_=ss_d[:, 512:S])
            nc.gpsimd.dma_start(out=ones_sb[:], in_=ones_d[:])
            nc.gpsimd.dma_start(out=tri_sb[:], in_=tri_d[:])

            # roped Q^T / K^T: 2 tiles each, rows = [headA(64) | headB(64)],
            # within each head block [x0(32) | x1(32)]
            qt = [singles.tile([128, S], BF16, tag=f"qt{m}", name=f"qt{m}") for m in range(2)]
            kt = [singles.tile([128, S], BF16, tag=f"kt{m}", name=f"kt{m}") for m in range(2)]
            # V with per-head ones column: head h occupies cols 65h..65h+63,
            # col 65h+64 is 1.0 (softmax denominator rides the PV matmul)
            NVT = S // 128
            v_sb = singles.tile([128, NVT, 4 * 65], BF16, tag="v")
            nc.vector.tensor_copy(
                v_sb.rearrange("p t (h e) -> p t h e", h=4)[:, :, :, 64:65],
                ones_sb.rearrange("p (t h) -> p t h", t=NVT)[:, :, :, None])
            # context per p-pair: rows [headA(64) | headB(64)]
            ctx2_sb = [singles.tile([128, S], BF16, tag=f"ctx{p}", name=f"ctx{p}")
                       for p in range(2)]

            with (
                tc.tile_pool(name="xin", bufs=4) as xin,
                tc.tile_pool(name="qkraw", bufs=3) as qkraw,
                tc.tile_pool(name="ropetmp", bufs=4) as ropetmp,
                tc.tile_pool(name="probs", bufs=6) as probspool,
                tc.tile_pool(name="recips", bufs=2) as recips,
                tc.tile_pool(name="ctxu", bufs=3) as ctxupool,
                tc.tile_pool(name="outsb", bufs=2) as outsb,
                tc.tile_pool(name="ps_pj", bufs=2, space="PSUM") as ps_pj,
                tc.tile_pool(name="ps_sc", bufs=4, space="PSUM") as ps_sc,
                tc.tile_pool(name="ps_ctx", bufs=2, space="PSUM") as ps_ctx,
            ):
                ps_bc = ps_sc       # pbc shares the ps_sc ring (same shape)

                # prefetch all x blocks up front: the sync queue later
                # carries AG-output reloads (which wait on collectives) and
                # must never have an x load queued behind one.
                xts = []
                for j in range(NQB):
                    xt_ = xin.tile([128, NKT, 512], BF16, name="xt")
                    nh = 4 if j == 0 else 1
                    for h in range(nh):
                        k0, k1 = (8 // nh) * h, (8 // nh) * (h + 1)
                        nc.sync.dma_start(
                            out=xt_[:, k0:k1, :],
                            in_=xT[128 * k0:128 * k1,
                                   512 * j:512 * (j + 1)].rearrange(
                                "(k p) n -> p k n", p=128))
                    xts.append(xt_)

                def emit_proj(j):
                    csl = slice(512 * j, 512 * (j + 1))
                    xt_ = xts[j]
                    # Q^T and K^T tiles: out (128 qdim, 512 tok)
                    for w_sb, raw_dst in ((wq_sb, qt), (wk_sb, kt)):
                        for m in range(2):
                            raw = qkraw.tile([128, 512], BF16, tag="raw", name="raw")
                            pq = ps_pj.tile([128, 512], F32, tag="pp", name="pq")
                            for k in range(NKT):
                                nc.tensor.matmul(
                                    pq[:], w_sb[:, k, 128 * m:128 * (m + 1)],
                                    xt_[:, k, :],
                                    start=(k == 0), stop=(k == NKT - 1))
                            nc.scalar.copy(out=raw[:], in_=pq[:])
                            # rope: dst = raw*cs + swap(raw)*ss; partner is
                            # 32 partitions away; 4 strided SBUF DMAs split
                            # across the gpsimd and scalar queues.
                            sw = ropetmp.tile([128, 512], BF16, tag="sw", name="sw")
                            for blk in range(2):
                                nc.gpsimd.dma_start(
                                    out=sw[64 * blk:64 * blk + 32, :],
                                    in_=raw[64 * blk + 32:64 * blk + 64, :])
                                nc.gpsimd.dma_start(
                                    out=sw[64 * blk + 32:64 * blk + 64, :],
                                    in_=raw[64 * blk:64 * blk + 32, :])
                            t1 = ropetmp.tile([128, 512], BF16, tag="t1", name="t1")
                            nc.vector.tensor_mul(t1[:], raw[:], cs_sb[:, csl])
                            nc.vector.tensor_mul(sw[:], sw[:], ss_sb[:, csl])
                            nc.vector.tensor_add(raw_dst[m][:, csl], t1[:], sw[:])
                    # V tiles: 2 token-tiles per PSUM bank, scattered into
                    # the 65-stride layout
                    for sp in range(2):
                        pv = ps_pj.tile([128, 512], F32, tag="pp", name="pv")
                        for i in range(2):
                            s_ = 2 * sp + i
                            for k in range(NKT):
                                nc.tensor.matmul(
                                    pv[:, 256 * i:256 * (i + 1)],
                                    xt_[:, k, 128 * s_:128 * (s_ + 1)],
                                    wv_sb[:, k, :],
                                    start=(k == 0), stop=(k == NKT - 1))
                        vt = 4 * j + 2 * sp
                        dst = v_sb[:, vt:vt + 2, :].rearrange(
                            "p t (h e) -> p t h e", h=4)[:, :, :, 0:64]
                        nc.vector.tensor_copy(
                            dst, pv[:].rearrange("p (t h e) -> p t h e", t=2, h=4))

                def emit_attn(qj, ps=(0, 1)):
                    qsl = slice(512 * qj, 512 * (qj + 1))
                    # diagonal chunks (windowed; i=0 covers the full 512 q so
                    # start=True initializes every PSUM element), then the
                    # fully-causal chunks in PAIRS sharing a bf16 PSUM bank.
                    diag_list = [(4 * qj + i, 128 * i) for i in range(4)]
                    off_list = list(range(4 * qj))
                    nunits = len(diag_list) + len(off_list)
                    for p in ps:
                        pctx = [ps_ctx.tile([65, 512], F32, tag="ctx", name="pctx")
                                for _ in range(2)]
                        ui = 0
                        for ch, w0 in diag_list:
                            pscs, prbs = [], []
                            for hh in range(2):
                                psc = ps_sc.tile([128, 512], F32, tag="ps", name="psc")
                                rsl = slice(64 * hh, 64 * (hh + 1))
                                nc.tensor.matmul(
                                    psc[:, w0:512],
                                    kt[p][rsl, 128 * ch:128 * (ch + 1)],
                                    qt[p][rsl, 512 * qj + w0:512 * (qj + 1)],
                                    start=True, stop=True)
                                pscs.append(psc)
                            for hh in range(2):
                                probs = probspool.tile([128, 512], BF16, tag="prb",
                                                       name="prb")
                                nc.scalar.activation(out=probs[:, w0:512],
                                                     in_=pscs[hh][:, w0:512],
                                                     func=AF.Exp, scale=inv_sqrt_dk)
                                sl = probs[:, w0:w0 + 128]
                                nc.vector.tensor_mul(sl, sl, tri_sb[:])
                                prbs.append(probs)
                            for hh in range(2):
                                h65 = 65 * (2 * p + hh)
                                nc.tensor.matmul(
                                    pctx[hh][:, w0:512],
                                    v_sb[:, ch, h65:h65 + 65],
                                    prbs[hh][:, w0:512],
                                    start=(ui == 0), stop=(ui == nunits - 1))
                            ui += 1
                        for ch in off_list:
                            pscs, prbs = [], []
                            for hh in range(2):
                                psc = ps_sc.tile([128, 512], F32, tag="ps", name="psc")
                                rsl = slice(64 * hh, 64 * (hh + 1))
                                nc.tensor.matmul(
                                    psc[:],
                                    kt[p][rsl, 128 * ch:128 * (ch + 1)],
                                    qt[p][rsl, qsl],
                                    start=True, stop=True)
                                pscs.append(psc)
                            for hh in range(2):
                                if hh == 1:
                                    pri = probspool.tile([128, 512], mybir.dt.int16,
                                                         tag="pri", name="pri")
                                    nc.vector.tensor_scalar(
                                        out=pri[:], in0=pscs[hh][:],
                                        scalar1=sch_a, scalar2=sch_b,
                                        op0=mybir.AluOpType.mult,
                                        op1=mybir.AluOpType.add)
                                    probs = pri[:].bitcast(BF16)
                                else:
                                    probs = probspool.tile([128, 512], BF16,
                                                           tag="prb", name="prb")
                                    nc.scalar.activation(out=probs[:],
                                                         in_=pscs[hh][:],
                                                         func=AF.Exp,
                                                         scale=inv_sqrt_dk)
                                prbs.append(probs)
                            for hh in range(2):
                                h65 = 65 * (2 * p + hh)
                                nc.tensor.matmul(
                                    pctx[hh][:, :],
                                    v_sb[:, ch, h65:h65 + 65],
                                    prbs[hh][:],
                                    start=False,
                                    stop=(ui == nunits - 1))
                            ui += 1
                        # normalize: denominator rode the PV matmul as row 64
                        recip = recips.tile([128, 1024], F32, tag="recip", name="recip")
                        recipr = recips.tile([128, 1024], BF16, tag="recipr", name="recipr")
                        ctxus = []
                        for hh in range(2):
                            ctxu = ctxupool.tile([65, 512], F32, tag="ctxu", name="ctxu")
                            nc.scalar.copy(out=ctxu[:], in_=pctx[hh][:])
                            nc.vector.reciprocal_approx_fast(
                                out=recip[0:65, 512 * hh:512 * (hh + 1)],
                                in_=ctxu[:])
                            ctxus.append(ctxu)
                        nc.vector.tensor_copy(recipr[64:65, :], recip[64:65, :])
                        for hh in range(2):
                            pbc = ps_bc.tile([128, 512], F32, tag="ps", name="pbc")[0:64, :]
                            nc.tensor.matmul(
                                pbc[:], ones_sb[64:65, 0:64],
                                recipr[64:65, 512 * hh:512 * (hh + 1)],
                                start=True, stop=True)
                            nc.vector.tensor_mul(
                                ctx2_sb[p][64 * hh:64 * (hh + 1), qsl],
                                ctxus[hh][0:64, :], pbc[:])

                def emit_outproj(qjo):
                    qsl = slice(512 * qjo, 512 * (qjo + 1))
                    stago = outsb.tile([128, 8, 512], BF16, tag="ot", name="stago")
                    for m in range(8):
                        po = ps_sc.tile([128, 512], F32, tag="ps", name="po")
                        for k in range(2):
                            nc.tensor.matmul(
                                po[:], wo_sb[:, k, 128 * m:128 * (m + 1)],
                                ctx2_sb[k][:, qsl],
                                start=(k == 0), stop=(k == 1))
                        # PSUM->SBUF bf16 cast copies: ACT (queue freed of
                        # swap DMAs), DVE takes every fourth for balance
                        if m % 4 == 3:
                            nc.vector.tensor_copy(stago[:, m, :], po[:])
                        else:
                            nc.scalar.copy(out=stago[:, m, :], in_=po[:])
                    for h in range(2):
                        nc.sync.dma_start(
                            out=out_d[512 * h:512 * (h + 1), qsl].rearrange(
                                "(m p) n -> p m n", p=128),
                            in_=stago[:, 4 * h:4 * (h + 1), :])

                # interleaved schedule: proj(j) unlocks attn(j); outproj
                # pipelined well behind its AG.
                emit_proj(0)
                emit_attn(0)
                emit_proj(1)
                emit_attn(1, ps=(0,))
                emit_outproj(0)
                emit_attn(1, ps=(1,))
                emit_proj(2)
                emit_attn(2, ps=(0,))
                emit_outproj(1)
                emit_attn(2, ps=(1,))
                emit_proj(3)
                emit_attn(3, ps=(0,))
                emit_outproj(2)
                emit_attn(3, ps=(1,))
                emit_outproj(3)

                if debug:
                    for m in range(2):
                        nc.sync.dma_start(out=dbg[f"dbg_qt{m}"][:], in_=qt[m][:])
                        nc.sync.dma_start(out=dbg[f"dbg_kt{m}"][:], in_=kt[m][:])
                    nc.sync.dma_start(out=dbg["dbg_v"][:],
                                      in_=v_sb.rearrange("p t e -> p (t e)"))
                    for p in range(2):
                        nc.sync.dma_start(out=dbg[f"dbg_ctx{p}"][:], in_=ctx2_sb[p][:])


def _build(debug: bool = False):
    nc = bacc.Bacc("TRN2", target_bir_lowering=False, debug=False, num_devices=NCORES)
    _emit(nc, debug=debug)
    nc.compile()
    return nc


def _perm_rows(g: int) -> np.ndarray:
    rows = []
    for l in range(HPC):
        h = HPC * g + l
        rows += [DK * h + d for d in range(0, DK, 2)]
        rows += [DK * h + d for d in range(1, DK, 2)]
    return np.asarray(rows)


def _wo_feat_perm(g: int) -> np.ndarray:
    # ctx2 slab feature order: slab p rows = [head 4g+2p (64) | head 4g+2p+1]
    perm = []
    for p in range(2):
        for hh in range(2):
            h = 4 * g + 2 * p + hh
            perm += [DK * h + d for d in range(DK)]
    return np.asarray(perm)


def kernel(x, token_positions, Wq, Wk, Wv, Wo):
    bf = mybir.dt.np(BF16)
    x = np.asarray(x, dtype=np.float32)
    Wq = np.asarray(Wq, dtype=np.float32)
    Wk = np.asarray(Wk, dtype=np.float32)
    Wv = np.asarray(Wv, dtype=np.float32)
    Wo = np.asarray(Wo, dtype=np.float32)
    pos = np.asarray(token_positions).astype(np.float64)

    debug = os.environ.get("KERNEL_DEBUG", "0") == "1"
    if "nc" not in _CACHE:
        _CACHE["nc"] = _build(debug=debug)
    nc = _CACHE["nc"]

    inv_freq = np.exp(np.arange(0, DK, 2, dtype=np.float32) * (-np.log(THETA) / DK)).astype(np.float64)
    ang = pos[:, None] * inv_freq[None, :]              # (S, 32)
    cos_t = np.cos(ang).astype(np.float32).T            # (32, S)
    sin_t = np.sin(ang).astype(np.float32).T
    fi = np.arange(128) % NF
    half = (np.arange(128) // NF) % 2
    CS = np.ascontiguousarray(cos_t[fi, :]).astype(bf)
    SS = np.ascontiguousarray(
        np.where(half[:, None] == 0, -sin_t[fi, :], sin_t[fi, :])).astype(bf)
    ONES = np.ones((128, 64), dtype=np.float32).astype(bf)
    TRI = np.triu(np.ones((128, 128), dtype=np.float32)).astype(bf)  # keep k<=q

    in_maps = []
    for c in range(NCORES):
        b, g = divmod(c, 4)
        pr = _perm_rows(g)
        in_maps.append({
            "xT": np.ascontiguousarray(x[b].T).astype(bf),
            "wqT": np.ascontiguousarray(Wq[pr].T).astype(bf),
            "wkT": np.ascontiguousarray(Wk[pr].T).astype(bf),
            "wvT": np.ascontiguousarray(Wv[GF * g:GF * (g + 1)].T).astype(bf),
            "woT": np.ascontiguousarray(Wo[:, _wo_feat_perm(g)].T).astype(bf),
            "cs": CS, "ss": SS, "ones": ONES, "tri": TRI,
        })

    trace = os.environ.get("KERNEL_TRACE", "0") == "1"
    res = run_bass_kernel_spmd(nc, in_maps, list(range(NCORES)), trace=trace)
    _CACHE["last_result"] = res

    acc = np.zeros((B, D, S), dtype=np.float32)
    for c in range(NCORES):
        b, g = divmod(c, 4)
        acc[b] += res.results[c]["out"].astype(np.float32)
    return np.ascontiguousarray(acc.transpose(0, 2, 1))
